# revision 1
# baseline (speedup 1.0000x reference)
"""Distributed AtomMessagePassing kernel for 8 TRN2 NeuronCores (Bass/Tile).

Strategy (dst-node sharding):
  - 50000 nodes split across 8 cores (6250 each); each edge owned by the core
    owning its dst, so the segment-sum stays core-local.
  - Algebraic restructure: concat([H[src], E]) @ W_h.T ==
        Adj @ (H @ W_hH.T)  +  (scatter_add(E, dst) @ W_hE.T + deg * b_h)
    so each layer is a per-edge dma_gather of premultiplied bf16 table rows +
    a one-hot matmul segment reduction on the TensorEngine (PSUM-accumulated
    per dst-block); the E-feature term folds into a precomputed bias.
  - Tables split top/bot so each half's AllGather overlaps compute; one-hot
    builds run in DVE bf16 2x mode; gather indices are SBUF-resident;
    bounce rows are written full-width so no zero-fill pass is needed.
  - Identical SPMD instruction stream; per-core variation is in input data.

Self-contained: hardcodes shapes; no sibling imports.
"""
import sys
sys.path.insert(0, '/opt/trn_rl_repo')
import numpy as np
import concourse.bass as bass
import concourse.mybir as mybir

F32 = mybir.dt.float32
BF16 = mybir.dt.bfloat16
I16 = mybir.dt.int16
RELU = mybir.ActivationFunctionType.Relu
EQ = mybir.AluOpType.is_equal
DW = 304  # on-chip per-block col width (32B aligned)


BLK = 128


def make_cfg(n_nodes=50000, d_v=133, d_e=14, d_h=300, gidx=1024, n_cores=8):
    nloc = n_nodes // n_cores
    assert nloc * n_cores == n_nodes
    nb = (nloc + BLK - 1) // BLK
    chunk = nb * BLK
    trows = chunk * n_cores
    # split dst-blocks into top/bot halves: separate tables so the AllGather
    # of each half can overlap compute. Row counts per half must be <= 32768.
    nbt = (nb + 1) // 2          # top blocks
    nbb = nb - nbt               # bot blocks
    ct, cb = nbt * BLK, nbb * BLK
    assert ct * n_cores <= 32768 and cb * n_cores <= 32768
    return dict(N_NODES=n_nodes, N_CORES=n_cores, NLOC=nloc, NB=nb, CHUNK=chunk,
                TROWS=trows, NBT=nbt, NBB=nbb, CT=ct, CB=cb,
                TOPR=ct * n_cores, BOTR=cb * n_cores, DPAD=384, GIDX=gidx,
                D_V=d_v, D_E=d_e, D_H=d_h)


def node_to_row(n, cfg):
    return (n // cfg['NLOC']) * cfg['CHUNK'] + (n % cfg['NLOC'])


def preprocess(edge_index, cfg):
    N_CORES, NLOC, NB = cfg['N_CORES'], cfg['NLOC'], cfg['NB']
    GIDX, CT, CB = cfg['GIDX'], cfg['CT'], cfg['CB']
    src = np.asarray(edge_index[0], dtype=np.int64)
    dst = np.asarray(edge_index[1], dtype=np.int64)
    core_of = dst // NLOC
    dloc = dst - core_of * NLOC
    blk = dloc // BLK
    sc = src // NLOC
    sl = src - sc * NLOC
    half = (sl >= CT).astype(np.int64)          # src in bot table?
    src_row = np.where(half == 0, sc * CT + sl, sc * CB + (sl - CT))

    counts = np.zeros((N_CORES, 2, NB), np.int64)
    lists = {}
    for c in range(N_CORES):
        mc = core_of == c
        for h in (0, 1):
            m = np.where(mc & (half == h))[0]
            order = np.lexsort((src[m], dloc[m]))
            m = m[order]
            bs = blk[m]
            cuts = np.searchsorted(bs, np.arange(NB + 1))
            for b in range(NB):
                e = m[cuts[b]:cuts[b + 1]]
                lists[(c, h, b)] = e
                counts[c, h, b] = len(e)

    pc = counts.max(axis=0)
    pc = ((pc + BLK - 1) // BLK) * BLK
    half_len = pc.sum(axis=1)
    half_pad = ((half_len + GIDX - 1) // GIDX) * GIDX
    nslots = int(half_pad.sum())
    nblk_tot = nslots // BLK

    sched = []
    slot_off = 0
    region_off = np.zeros((2, NB), np.int64)
    for h in (0, 1):
        h_start = slot_off
        for b in range(NB):
            region_off[h, b] = slot_off
            sched.append(dict(h=h, b=b, off=int(slot_off),
                              nblk=int(pc[h, b] // BLK)))
            slot_off += int(pc[h, b])
        slot_off = h_start + int(half_pad[h])
    assert slot_off == nslots

    cores = []
    for c in range(N_CORES):
        idx_slots = np.zeros(nslots, np.int16)
        rel_slots = np.full(nslots, -1.0, np.float32)
        eidx_slots = np.full(nslots, -1, np.int64)
        for h in (0, 1):
            for b in range(NB):
                e = lists[(c, h, b)]
                o = int(region_off[h, b])
                idx_slots[o:o + len(e)] = src_row[e].astype(np.int16)
                rel_slots[o:o + len(e)] = (dloc[e] - b * BLK).astype(np.float32)
                eidx_slots[o:o + len(e)] = e
        cores.append(dict(idx=idx_slots, rel=rel_slots, eidx=eidx_slots))

    meta = dict(nslots=nslots, nblk_tot=nblk_tot, sched=sched,
                half_pad=[int(x) for x in half_pad], pc=pc, counts=counts)
    return cores, meta

def _patch_tile():
    """walrus in this container rejects Drain instructions with >1 sem wait;
    offload excess waits onto preceding nops."""
    from concourse.tile import TileContext, ScopedClock
    if getattr(TileContext, "_drain_patched", False):
        return

    def _drain_and_barrier(self, tick_clock, wait_clock):
        drain_inst = self.nc.sync.drain()
        wait_clock.add_sem_waits(
            drain_inst.ins, ScopedClock({None: tick_clock.global_clock}))
        si = drain_inst.ins.sync_info
        if si is not None and si.on_wait and len(si.on_wait) > 1:
            waits = list(si.on_wait)
            keep, excess = waits[:1], waits[1:]
            bb = self.nc.cur_bb.bb
            insts = bb.instructions
            assert insts[-1] is drain_inst.ins
            insts.pop()
            for w in excess:
                nop = self.nc.sync.nop(nofuse=True, hint="drain_wait_split")
                if nop.ins.sync_info is None:
                    nop.ins.sync_info = mybir.SyncInfo(on_wait=[w], on_update=[])
                else:
                    nop.ins.sync_info.on_wait.append(w)
            si.on_wait.clear()
            for w in keep:
                si.on_wait.append(w)
            bb.add_instruction(drain_inst.ins)

        self.nc.all_engine_barrier()
        assert self.sems is not None
        popped = self.nc._tile_sem_poison_stack.pop()
        assert popped is self._sem_poison
        self.nc.clear_and_free_semaphores(list(self.sems.allocated().values()))
        self.nc.all_engine_barrier()

    TileContext._drain_and_barrier = _drain_and_barrier
    TileContext._drain_patched = True




PREDICTED_NS = None

def build_kernel(cfg, meta, no_coll=False, gbufs=4, sbufs=8):
    global PREDICTED_NS
    _patch_tile()
    NLOC, NB, CHUNK = cfg['NLOC'], cfg['NB'], cfg['CHUNK']
    TROWS, DPAD, GIDX = cfg['TROWS'], cfg['DPAD'], cfg['GIDX']
    NBT, CT, CB = cfg['NBT'], cfg['CT'], cfg['CB']
    TOPR, BOTR = cfg['TOPR'], cfg['BOTR']
    D_V, D_H, D_E = cfg['D_V'], cfg['D_H'], cfg['D_E']
    assert CHUNK == NB * BLK and TROWS == 8 * CHUNK
    nslots = meta['nslots']
    ninstr = nslots // GIDX
    ipg = GIDX // BLK
    half_pad = list(meta['half_pad'])
    assert half_pad[0] % GIDX == 0 and half_pad[1] % GIDX == 0
    ninstr_h0 = half_pad[0] // GIDX
    nblk_tot = nslots // BLK
    IW = GIDX // 16

    blocks = [None] * nblk_tot
    for r in meta['sched']:
        for k in range(r['nblk']):
            gb = r['off'] // BLK + k
            blocks[gb] = dict(b=r['b'], h=r['h'], first=(k == 0),
                              last=(k == r['nblk'] - 1))
    regions = {(r['h'], r['b']): r for r in meta['sched']}

    from concourse.tile import TileContext
    from concourse.bacc import Bacc

    entries_box = []
    orig_exit = TileContext.__exit__

    def patched_exit(self2, *a):
        r = orig_exit(self2, *a)
        entries_box.append(list(getattr(self2, "_perfetto_entries", []) or []))
        TileContext.__exit__ = orig_exit
        return r

    TileContext.__exit__ = patched_exit

    nc = Bacc(num_devices=8)

    def Par(name, shape, dt):
        return nc.declare_dram_parameter(name, shape, dt, isOutput=False)

    vt_a = Par("vt_a", [128, CHUNK], BF16)
    vt_b = Par("vt_b", [6, CHUNK], BF16)
    wi_a = Par("wi_a", [128, D_H], BF16)
    wi_b = Par("wi_b", [6, D_H], BF16)
    a0 = Par("a0", [128, D_H], BF16)
    a1 = Par("a1", [128, D_H], BF16)
    a2 = Par("a2", [48, D_H], BF16)
    wov_a = Par("wov_a", [128, D_H], BF16)
    wov_b = Par("wov_b", [6, D_H], BF16)
    wom0 = Par("wom0", [128, D_H], BF16)
    wom1 = Par("wom1", [128, D_H], BF16)
    wom2 = Par("wom2", [48, D_H], BF16)
    wfull = Par("wfull", [16, D_H], BF16)
    iota_p = Par("iota", [128, 128], BF16)
    ident_p = Par("ident", [128, 128], F32)
    identb_p = Par("identb", [128, 128], BF16)
    idx_p = Par("idx", [128, ninstr * IW], I16)
    rel_p = Par("rel", [128, nblk_tot], F32)
    eperm_p = Par("eperm", [128, nblk_tot * 16], BF16)
    out_p = nc.declare_dram_parameter("out", [NLOC, D_H], F32, isOutput=True)

    with TileContext(nc) as tc:
        with (
            tc.tile_pool(name="const", bufs=1) as constp,
            tc.tile_pool(name="bigsb", bufs=1) as bigp,
            tc.tile_pool(name="gpool", bufs=gbufs) as gpool,
            tc.tile_pool(name="idxp", bufs=3) as idxp,
            tc.tile_pool(name="spool", bufs=sbufs) as spool,
            tc.tile_pool(name="htp", bufs=6) as htp,
            tc.tile_pool(name="misc", bufs=3) as miscp,
            tc.tile_pool(name="ep", bufs=2) as epool,
            tc.tile_pool(name="psP", bufs=2, space="PSUM") as psP,
            tc.tile_pool(name="psC", bufs=2, space="PSUM") as psC,
            tc.tile_pool(name="psT", bufs=2, space="PSUM") as psT,
            tc.tile_pool(name="psX", bufs=2, space="PSUM") as psX,
            tc.tile_pool(name="dram", bufs=1, space="DRAM") as dramp,
        ):
            vt_a_sb = constp.tile([128, CHUNK], BF16, name="vt_a_sb")
            vt_b_sb = constp.tile([6, CHUNK], BF16, name="vt_b_sb")
            wi_a_sb = constp.tile([128, D_H], BF16, name="wi_a_sb")
            wi_b_sb = constp.tile([6, D_H], BF16, name="wi_b_sb")
            a0_sb = constp.tile([128, D_H], BF16, name="a0_sb")
            a1_sb = constp.tile([128, D_H], BF16, name="a1_sb")
            a2_sb = constp.tile([48, D_H], BF16, name="a2_sb")
            wov_a_sb = constp.tile([128, D_H], BF16, name="wov_a_sb")
            wov_b_sb = constp.tile([6, D_H], BF16, name="wov_b_sb")
            wom0_sb = constp.tile([128, D_H], BF16, name="wom0_sb")
            wom1_sb = constp.tile([128, D_H], BF16, name="wom1_sb")
            wom2_sb = constp.tile([48, D_H], BF16, name="wom2_sb")
            wfull_sb = constp.tile([16, D_H], BF16, name="wfull_sb")
            iota_sb = constp.tile([128, 128], BF16, name="iota_sb")
            ident_sb = constp.tile([128, 128], F32, name="ident_sb")
            identb_sb = constp.tile([128, 128], BF16, name="identb_sb")
            rel_sb = constp.tile([128, nblk_tot], F32, name="rel_sb")
            idx_sb = constp.tile([128, ninstr * IW], I16, name="idx_sb")
            B_sb = bigp.tile([128, NB * DW], F32, name="B_sb")
            Pt_sb = bigp.tile([128, NB * DW], F32, name="Pt_sb")

            for dst, src in [(vt_b_sb, vt_b), (wi_a_sb, wi_a),
                             (wi_b_sb, wi_b), (a0_sb, a0), (a1_sb, a1),
                             (a2_sb, a2), (wov_a_sb, wov_a), (wov_b_sb, wov_b),
                             (wom0_sb, wom0), (wom1_sb, wom1), (wom2_sb, wom2),
                             (wfull_sb, wfull), (iota_sb, iota_p),
                             (ident_sb, ident_p), (identb_sb, identb_p),
                             (rel_sb, rel_p), (idx_sb, idx_p)]:
                nc.sync.dma_start(out=dst[:, :], in_=src[:, :])
            # vt_a in per-stage0-block-group chunks so block 0 starts early
            vt_step = 7 * BLK
            for o in range(0, CHUNK, vt_step):
                w = min(vt_step, CHUNK - o)
                nc.sync.dma_start(out=vt_a_sb[:, o:o + w], in_=vt_a[:, o:o + w])

            # one shared register for every gather's num_idxs
            gidx_reg = nc.gpsimd.to_reg(GIDX)

            bounce_t = dramp.tile([CT, DPAD], BF16, name="bounce_t")
            bounce_b = dramp.tile([CB, DPAD], BF16, name="bounce_b")
            toptabs = [dramp.tile([TOPR, DPAD], BF16, name=f"toptab{t}",
                                  addr_space="Shared") for t in range(3)]
            bottabs = [dramp.tile([BOTR, DPAD], BF16, name=f"bottab{t}",
                                  addr_space="Shared") for t in range(3)]

            # zero pad columns of B/Pt so transposed garbage can't be NaN
            nc.vector.memset(B_sb[:, :], 0.0)

            def bounce_rows(b):
                # full-width rows: finalize tiles carry zeroed pad columns, so
                # every bounce row is fully written each layer (no zero-fill)
                if b < NBT:
                    return bounce_t[b * BLK:(b + 1) * BLK, :]
                bb = b - NBT
                return bounce_b[bb * BLK:(bb + 1) * BLK, :]

            def emit_allgather(t):
                nc.gpsimd.collective_compute(
                    "AllGather", mybir.AluOpType.bypass,
                    replica_groups=[list(range(8))],
                    ins=[bounce_t[:, :]], outs=[toptabs[t][:, :]])
                nc.gpsimd.collective_compute(
                    "AllGather", mybir.AluOpType.bypass,
                    replica_groups=[list(range(8))],
                    ins=[bounce_b[:, :]], outs=[bottabs[t][:, :]])

            def transpose3(col_ap_fn, dtype_in=F32):
                outs = []
                for k in range(3):
                    w = 128 if k < 2 else 48
                    tp = psT.tile([w, 128], F32, name="tp", tag="tp")
                    ident = ident_sb if dtype_in == F32 else identb_sb
                    nc.tensor.matmul(tp[:, :], col_ap_fn(k), ident[:, :],
                                     start=True, stop=True, is_transpose=True)
                    ht = htp.tile([w, 128], BF16, name="ht", tag="ht")
                    nc.scalar.copy(ht[:, :], tp[:, :])
                    outs.append(ht)
                return outs

            def bcol(b, k):
                w = 128 if k < 2 else 48
                return B_sb[:, b * DW + 128 * k: b * DW + 128 * k + w]

            def ptcol(b, k):
                w = 128 if k < 2 else 48
                return Pt_sb[:, b * DW + 128 * k: b * DW + 128 * k + w]

            # ============== C phase body (E reduce, per block) ==============
            def c_phase_block(b):
                regs = [regions[(h, b)] for h in (0, 1) if regions[(h, b)]['nblk'] > 0]
                tot = sum(r['nblk'] for r in regs)
                if tot == 0:
                    return
                cps = psC.tile([128, 16], F32, name="cps", tag="cps")
                done = 0
                for r in regs:
                    gb0 = r['off'] // BLK
                    et = epool.tile([128, r['nblk'] * 16], BF16, name="et", tag="et")
                    nc.sync.dma_start(out=et[:, :],
                                      in_=eperm_p[:, gb0 * 16:(gb0 + r['nblk']) * 16])
                    for k in range(r['nblk']):
                        gb = gb0 + k
                        s = spool.tile([128, 128], BF16, name="s", tag="s")
                        nc.vector.tensor_scalar(s[:, :], iota_sb[:, :],
                                                rel_sb[:, gb:gb + 1], None, op0=EQ)
                        nc.tensor.matmul(cps[:, :], s[:, :], et[:, k * 16:(k + 1) * 16],
                                         start=(done == 0), stop=(done == tot - 1))
                        done += 1
                eb = miscp.tile([128, 16], BF16, name="eb", tag="eb")
                nc.scalar.copy(eb[:, :], cps[:, :])
                etp = psT.tile([16, 128], BF16, name="etp", tag="tp")
                nc.tensor.matmul(etp[:, :], eb[:, :], identb_sb[:, :],
                                 start=True, stop=True, is_transpose=True)
                ets = htp.tile([16, 128], BF16, name="ets", tag="ht")
                nc.scalar.copy(ets[:, :], etp[:, :])
                c2 = psC.tile([128, D_H], F32, name="c2", tag="cps")
                nc.tensor.matmul(c2[:, :], ets[:, :], wfull_sb[:, :],
                                 start=True, stop=True)
                nc.vector.tensor_add(B_sb[:, b * DW:b * DW + D_H],
                                     B_sb[:, b * DW:b * DW + D_H], c2[:, :])

            # ======================= stage 0: H0 + X0 (+C) =======================
            for b in range(NB):
                bsl = slice(b * BLK, (b + 1) * BLK)
                h0 = psX.tile([128, D_H], F32, name="h0", tag="px")
                nc.tensor.matmul(h0[:, :], vt_a_sb[:, bsl], wi_a_sb[:, :],
                                 start=True, stop=False)
                nc.tensor.matmul(h0[:, :], vt_b_sb[:, bsl], wi_b_sb[:, :],
                                 start=False, stop=True)
                nc.scalar.activation(B_sb[:, b * DW:b * DW + D_H], h0[:, :], RELU)
                hts = transpose3(lambda k: bcol(b, k))
                x0 = psX.tile([128, D_H], F32, name="x0", tag="px")
                nc.tensor.matmul(x0[:, :], hts[0][:, :], a0_sb[:, :], start=True, stop=False)
                nc.tensor.matmul(x0[:, :], hts[1][:, :], a1_sb[:, :], start=False, stop=False)
                nc.tensor.matmul(x0[:, :], hts[2][:, :], a2_sb[:, :], start=False, stop=True)
                xb = miscp.tile([128, DPAD], BF16, name="xb", tag="xb")
                nc.scalar.copy(xb[:, 0:D_H], x0[:, :])
                nc.vector.memset(xb[:, D_H:DPAD], 0.0)
                nc.sync.dma_start(out=bounce_rows(b), in_=xb[:, :])
                c_phase_block(b)

            if not no_coll:
                emit_allgather(0)

            # ======================= per-block finalize =======================
            def finalize_block(layer, b):
                bsl = slice(b * BLK, (b + 1) * BLK)
                if layer < 3:
                    # H = relu(B + Pt) ; write H into Pt_sb[b] (f32)
                    nc.vector.tensor_add(Pt_sb[:, b * DW:b * DW + D_H],
                                         Pt_sb[:, b * DW:b * DW + D_H],
                                         B_sb[:, b * DW:b * DW + D_H])
                    nc.scalar.activation(Pt_sb[:, b * DW:b * DW + D_H],
                                         Pt_sb[:, b * DW:b * DW + D_H], RELU)
                    if layer == 2:
                        # raw H2 -> bounce
                        hb = miscp.tile([128, DPAD], BF16, name="hb", tag="xb")
                        nc.vector.tensor_copy(hb[:, 0:D_H], Pt_sb[:, b * DW:b * DW + D_H])
                        nc.vector.memset(hb[:, D_H:DPAD], 0.0)
                        nc.sync.dma_start(out=bounce_rows(b), in_=hb[:, :])
                    else:
                        hts = transpose3(lambda k: ptcol(b, k))
                        x1 = psX.tile([128, D_H], F32, name="x1", tag="px")
                        nc.tensor.matmul(x1[:, :], hts[0][:, :], a0_sb[:, :], start=True, stop=False)
                        nc.tensor.matmul(x1[:, :], hts[1][:, :], a1_sb[:, :], start=False, stop=False)
                        nc.tensor.matmul(x1[:, :], hts[2][:, :], a2_sb[:, :], start=False, stop=True)
                        xb = miscp.tile([128, DPAD], BF16, name="xb", tag="xb")
                        nc.scalar.copy(xb[:, 0:D_H], x1[:, :])
                        nc.vector.memset(xb[:, D_H:DPAD], 0.0)
                        nc.sync.dma_start(out=bounce_rows(b), in_=xb[:, :])
                else:
                    # out = relu(V@WoV + Mv@WoM + b_o), Mv = Pt
                    hts = transpose3(lambda k: ptcol(b, k))
                    hv = psX.tile([128, D_H], F32, name="hv", tag="px")
                    nc.tensor.matmul(hv[:, :], vt_a_sb[:, bsl], wov_a_sb[:, :],
                                     start=True, stop=False)
                    nc.tensor.matmul(hv[:, :], vt_b_sb[:, bsl], wov_b_sb[:, :],
                                     start=False, stop=False)
                    nc.tensor.matmul(hv[:, :], hts[0][:, :], wom0_sb[:, :],
                                     start=False, stop=False)
                    nc.tensor.matmul(hv[:, :], hts[1][:, :], wom1_sb[:, :],
                                     start=False, stop=False)
                    nc.tensor.matmul(hv[:, :], hts[2][:, :], wom2_sb[:, :],
                                     start=False, stop=True)
                    ob = miscp.tile([128, D_H], F32, name="ob", tag="ob")
                    nc.scalar.activation(ob[:, :], hv[:, :], RELU)
                    lo = b * BLK
                    hi = min(NLOC, (b + 1) * BLK)
                    nc.sync.dma_start(out=out_p[lo:hi, :], in_=ob[0:hi - lo, :])

            # ======================= layers =======================
            for layer in (1, 2, 3):
                ttab, btab = toptabs[layer - 1], bottabs[layer - 1]
                nc.vector.memset(Pt_sb[:, :], 0.0)
                open_psum = {}
                for j in range(ninstr):
                    h = 0 if j < ninstr_h0 else 1
                    g = gpool.tile([128, ipg, DPAD], BF16, name="g", tag="g")
                    nc.gpsimd.dma_gather(
                        out_ap=g[:, :, :],
                        in_ap=(ttab if h == 0 else btab)[:, :],
                        idxs_ap=idx_sb[:, j * IW:(j + 1) * IW],
                        num_idxs=GIDX,
                        num_idxs_reg=gidx_reg,
                        elem_size=DPAD,
                    )
                    for k in range(ipg):
                        gb = j * ipg + k
                        info = blocks[gb]
                        if info is None:
                            continue
                        b = info['b']
                        s = spool.tile([128, 128], BF16, name="s", tag="s")
                        nc.vector.tensor_scalar(s[:, :], iota_sb[:, :],
                                                rel_sb[:, gb:gb + 1], None, op0=EQ)
                        if info['first']:
                            open_psum[b] = psP.tile([128, D_H], F32, name="pp", tag="pp")
                        pp = open_psum[b]
                        nc.tensor.matmul(pp[:, :], s[:, :], g[:, k, 0:D_H],
                                         start=info['first'], stop=info['last'])
                        if info['last']:
                            nc.vector.tensor_add(
                                Pt_sb[:, b * DW:b * DW + D_H],
                                Pt_sb[:, b * DW:b * DW + D_H], pp[:, :])
                            del open_psum[b]
                            if info['h'] == 1 or regions[(1, b)]['nblk'] == 0:
                                finalize_block(layer, b)
                for b in range(NB):
                    if regions[(0, b)]['nblk'] == 0 and regions[(1, b)]['nblk'] == 0:
                        finalize_block(layer, b)
                if layer < 3 and not no_coll:
                    emit_allgather(layer)

    nc.compile()
    if entries_box and entries_box[0]:
        ent = entries_box[0]
        starts = [e[1] for e in ent if e[1] is not None]
        ends = [e[2] for e in ent if len(e) > 2 and e[2] is not None]
        if starts and ends:
            PREDICTED_NS = int(max(ends) - min(starts))
    return nc


def host_arrays(cfg, meta, cores_prep, V, E, W_i, b_i, W_h, b_h, W_o, b_o):
    """Build per-core in_maps. cores_prep: from prep.preprocess."""
    import ml_dtypes
    BF = ml_dtypes.bfloat16
    NLOC, NB, CHUNK = cfg['NLOC'], cfg['NB'], cfg['CHUNK']
    GIDX, D_V, D_H, D_E = cfg['GIDX'], cfg['D_V'], cfg['D_H'], cfg['D_E']
    nslots = meta['nslots']
    ninstr = nslots // GIDX
    IW = GIDX // 16
    nblk_tot = nslots // BLK

    A_pad = np.zeros((304, D_H), np.float32)
    A_pad[:D_H] = W_h[:, :D_H].T
    WiT = np.concatenate([W_i.T, b_i[None, :]], 0)           # [134, 300]
    WoV = np.concatenate([W_o[:, :D_V].T, b_o[None, :]], 0)  # [134, 300]
    WoM_pad = np.zeros((304, D_H), np.float32)
    WoM_pad[:D_H] = W_o[:, D_V:].T
    Wfull = np.zeros((16, D_H), np.float32)
    Wfull[:D_E] = W_h[:, D_H:].T
    Wfull[D_E] = b_h

    iota = np.broadcast_to(np.arange(128, dtype=np.float32), (128, 128)).copy()
    ident = np.eye(128, dtype=np.float32)

    shared = dict(
        wi_a=WiT[0:128].astype(BF), wi_b=WiT[128:134].astype(BF),
        a0=A_pad[0:128].astype(BF), a1=A_pad[128:256].astype(BF),
        a2=A_pad[256:304].astype(BF),
        wov_a=WoV[0:128].astype(BF), wov_b=WoV[128:134].astype(BF),
        wom0=WoM_pad[0:128].astype(BF), wom1=WoM_pad[128:256].astype(BF),
        wom2=WoM_pad[256:304].astype(BF),
        wfull=Wfull.astype(BF),
        iota=iota.astype(BF), ident=ident, identb=ident.astype(BF),
    )
    assert shared['wi_b'].shape[0] == 6 and shared['wov_b'].shape[0] == 6

    in_maps = []
    for c in range(8):
        cp = cores_prep[c]
        # V^T chunk + ones row, padded to CHUNK cols
        Vc = V[c * NLOC:(c + 1) * NLOC]                      # [NLOC, 133]
        vt = np.zeros((134, CHUNK), np.float32)
        vt[0:D_V, 0:NLOC] = Vc.T
        vt[133, 0:NLOC] = 1.0
        # idx: per-instruction 16-wrap layout, replicated to 128 partitions
        idx = cp['idx']                                      # [nslots] int16
        idxw = np.zeros((16, ninstr * IW), np.int16)
        s = np.arange(nslots)
        j = s // GIDX
        i = s % GIDX
        idxw[i % 16, j * IW + i // 16] = idx
        idx128 = np.tile(idxw, (8, 1))
        # rel: gather layout [128, nblk_tot]
        rel = cp['rel'].reshape(nblk_tot, BLK).T.copy()
        # eperm: [128, nblk_tot*16] (E rows + ones col in slot order)
        ep = np.zeros((nslots, 16), np.float32)
        m = cp['eidx'] >= 0
        ep[m, 0:D_E] = E[cp['eidx'][m]]
        ep[m, D_E] = 1.0
        ep = ep.reshape(nblk_tot, BLK, 16).transpose(1, 0, 2).reshape(128, nblk_tot * 16)
        in_maps.append(dict(
            vt_a=vt[0:128].astype(BF), vt_b=vt[128:134].astype(BF),
            idx=idx128, rel=rel.astype(np.float32), eperm=ep.astype(BF),
            **{k: v.copy() for k, v in shared.items()},
        ))
    return in_maps


# --------------------------------------------------------------------------
# entry point
# --------------------------------------------------------------------------
TRACE = False
LAST_EXEC_NS = None


def kernel(V, E, edge_index, W_i, b_i, W_h, b_h, W_o, b_o):
    global LAST_EXEC_NS
    from concourse.bass_utils import run_bass_kernel_spmd

    V = np.asarray(V, np.float32)
    E = np.asarray(E, np.float32)
    edge_index = np.asarray(edge_index)
    W_i = np.asarray(W_i, np.float32)
    b_i = np.asarray(b_i, np.float32)
    W_h = np.asarray(W_h, np.float32)
    b_h = np.asarray(b_h, np.float32)
    W_o = np.asarray(W_o, np.float32)
    b_o = np.asarray(b_o, np.float32)

    cfg = make_cfg(n_nodes=V.shape[0], d_v=V.shape[1], d_e=E.shape[1],
                   d_h=W_i.shape[0])
    cores_prep, meta = preprocess(edge_index, cfg)
    nc = build_kernel(cfg, meta)
    in_maps = host_arrays(cfg, meta, cores_prep, V, E, W_i, b_i, W_h, b_h,
                          W_o, b_o)
    kw = {}
    if TRACE:
        import tempfile
        kw = dict(trace=True, tmpdir=tempfile.mkdtemp(prefix="gnn_trace_"))
    res = run_bass_kernel_spmd(nc, in_maps, core_ids=list(range(8)), **kw)
    LAST_EXEC_NS = res.exec_time_ns
    out = np.concatenate([res.results[i]["out"] for i in range(8)], 0)
    return out[:V.shape[0]].astype(np.float32)

